# revision 1
# baseline (speedup 1.0000x reference)
"""Trainium2 Bass kernel for nn_AttnRes: 8-layer attn/MLP net with depth-
aggregation over a history buffer.

Sharding: pure data-parallel over B — each of the 8 NeuronCores runs the full
L=8 layer network on one batch element [T=1024, D=768]. No collectives.

Layouts per core:
  partial (residual accumulator): SBUF f32, 8 t-tiles [128, 768]
  hist entries: DRAM bf16 [1024, 768] (+ cached proj-dots [t,16] and sum-sq)
  per-layer hT (rms-normed aggregate, transposed): 6 d-tiles [128, 1024] bf16
  attention: scores/expP computed transposed [k, q] so softmax needs no
  max-subtraction and the PV matmul needs no transposes; denominators come
  from an appended ones-column in the value matrix.
All matmuls bf16 with f32 PSUM accumulation; the residual stream stays f32.
attn_scale/mlp_scale and the 1/sqrt(HD) factor are folded into Wo/Wm2/Wq on
the host; softmax and rms-norm normalizers are folded algebraically.
"""

import os
import sys

sys.path.insert(0, "/opt/trn_rl_repo")

_SKIP = os.environ.get("KSKIP", "")

import numpy as np
import ml_dtypes

import concourse.bass as bass
import concourse.tile as tile
import concourse.mybir as mybir
from concourse.bass_utils import run_bass_kernel_spmd
from concourse.library_overlay import lower_extended_insts

BF = mybir.dt.bfloat16
F32 = mybir.dt.float32
AX = mybir.AxisListType
ALU = mybir.AluOpType
ACTF = mybir.ActivationFunctionType

T, D, H, HD, L = 1024, 768, 12, 64, 8
DB = D // 128          # 6 d-blocks
TT = T // 128          # 8 t-tiles
CK = (4 * D) // 128    # 24 mlp c-tiles
EPS = float(np.finfo(np.float32).eps)
LN2 = float(np.log(2.0))

_CACHE = {}
LAST_RESULT = None


def _hoist_waits(nc, max_keep=1):
    """Engine-instruction ISA structs encode at most ~1 semaphore wait;
    move excess waits onto same-engine NoOps inserted just before."""
    f = nc.m.functions[0]
    for blk in f.blocks:
        new = []
        for inst in blk.instructions:
            si = inst.sync_info
            if (
                si is not None
                and si.on_wait
                and len(si.on_wait) > max_keep
                and inst.engine != mybir.EngineType.Unassigned
            ):
                waits = list(si.on_wait)
                extra, keep = waits[:-max_keep], waits[-max_keep:]
                for k, w in enumerate(extra):
                    nop = mybir.InstNoOp(name=f"{inst.name}hw{k}", ins=[], outs=[])
                    nop.engine = inst.engine
                    nop.sync_info = mybir.SyncInfo(on_wait=[w], on_update=[])
                    new.append(nop)
                inst.sync_info = mybir.SyncInfo(
                    on_wait=keep, on_update=list(si.on_update or [])
                )
            new.append(inst)
        blk.instructions = new


def build(ln_s):
    nc = bass.Bass()

    x_d = nc.declare_dram_parameter("x", [T, D], F32, isOutput=False)
    wqk_d = nc.declare_dram_parameter("wqk", [L, D, 2 * D], BF, isOutput=False)  # cols: g3*512 + (q:0-255 | k:256-511)
    wv_d = nc.declare_dram_parameter("wv", [L, D, D], BF, isOutput=False)
    wo_d = nc.declare_dram_parameter("wo", [L, D, D], BF, isOutput=False)
    wm1_d = nc.declare_dram_parameter("wm1", [L, D, 4 * D], BF, isOutput=False)
    wm2_d = nc.declare_dram_parameter("wm2", [L, 4 * D, D], BF, isOutput=False)
    pall_d = nc.declare_dram_parameter("pall", [128, DB * 16], BF, isOutput=False)
    pallt_d = nc.declare_dram_parameter("pallt", [16, D], F32, isOutput=False)
    mask_d = nc.declare_dram_parameter("maskt", [128, 128], F32, isOutput=False)
    idf_d = nc.declare_dram_parameter("idf", [128, 128], F32, isOutput=False)
    idb_d = nc.declare_dram_parameter("idb", [128, 128], BF, isOutput=False)
    out_d = nc.declare_dram_parameter("out", [T, D], F32, isOutput=True)

    with tile.TileContext(nc) as tc:
        with (
            tc.tile_pool(name="consts", bufs=1) as consts,
            tc.tile_pool(name="persist", bufs=1) as persist,
            tc.tile_pool(name="wpool", bufs=1) as wpool,
            tc.tile_pool(name="wm2p", bufs=4) as wm2p,
            tc.tile_pool(name="work", bufs=2) as work,
            tc.tile_pool(name="small", bufs=3) as small,
            tc.tile_pool(name="hep", bufs=4) as hep,
            tc.tile_pool(name="diagp", bufs=4) as diagp,
            tc.tile_pool(name="qkp", bufs=2) as qkp,
            tc.tile_pool(name="vap", bufs=1) as vap,
            tc.tile_pool(name="expp", bufs=1) as expp,
            tc.tile_pool(name="atp", bufs=1) as atp,
            tc.tile_pool(name="htp", bufs=1) as htp,
            tc.tile_pool(name="pbp", bufs=3) as pbp,
            tc.tile_pool(name="m1p", bufs=4) as m1p,
            tc.tile_pool(name="dtp", bufs=2) as dtp,
            tc.tile_pool(name="ps", bufs=8, space="PSUM") as psp,
            tc.tile_pool(name="dramp", bufs=1, space="DRAM") as dramp,
            tc.tile_pool(name="drbp", bufs=4, space="DRAM") as drbp,
        ):
            ctr = [0]

            def uname(pfx):
                ctr[0] += 1
                return f"{pfx}{ctr[0]}"

            def psum(p, n, dt=F32):
                return psp.tile([p, n], dt, tag="ps", name=uname("ps"))

            # ---- constants ----
            mask_s = consts.tile([128, 128], F32, tag="mask")
            nc.sync.dma_start(out=mask_s, in_=mask_d[:, :])
            idf_s = consts.tile([128, 128], F32, tag="idf")
            nc.sync.dma_start(out=idf_s, in_=idf_d[:, :])
            idb_s = consts.tile([128, 128], BF, tag="idb")
            nc.sync.dma_start(out=idb_s, in_=idb_d[:, :])
            c_zero = consts.tile([128, 1], F32, tag="c_zero")
            nc.vector.memset(c_zero, 0.0)
            c_eps = consts.tile([128, 1], F32, tag="c_eps")
            nc.vector.memset(c_eps, EPS)
            ones_r = consts.tile([1, 64], BF, tag="ones_r")
            nc.vector.memset(ones_r, 1.0)
            pall_s = consts.tile([128, DB * 16], BF, tag="pall")
            nc.sync.dma_start(out=pall_s, in_=pall_d[:, :])

            # ---- persistent state ----
            PT = persist.tile([128, TT * D], F32, tag="PT", name="PT")
            pt = [PT[:, tt * D:(tt + 1) * D] for tt in range(TT)]
            dots = [persist.tile([128, TT * 16], F32, tag=f"dots{e}", name=f"dots{e}") for e in range(4)]
            ssqs = [persist.tile([128, TT], F32, tag=f"ssq{e}", name=f"ssq{e}") for e in range(4)]
            # hist entries in DRAM (bf16), tracked by the tile framework
            edram = [dramp.tile([T, D], BF, tag=f"hist{e}", name=f"hist{e}") for e in range(4)]

            for tt in range(TT):
                nc.sync.dma_start(out=pt[tt], in_=x_d[tt * 128:(tt + 1) * 128, :])

            def do_commit(eidx):
                """Snapshot current partial as history entry eidx: store bf16
                copy to DRAM, cache sum-of-squares and projection dots."""
                for tt in range(TT):
                    trash = work.tile([128, D], BF, tag="trash")
                    nc.scalar.activation(
                        trash, pt[tt], ACTF.Square, bias=c_zero,
                        accum_out=ssqs[eidx][:, tt:tt + 1],
                    )
                    pb = pbp.tile([128, D], BF, tag="pbf")
                    nc.vector.tensor_copy(pb, pt[tt])
                    nc.sync.dma_start(
                        out=edram[eidx][tt * 128:(tt + 1) * 128, :], in_=pb
                    )
                    # transpose the bf16 copy via DMA xbar, then PE dots
                    dps = psum(128, 16)
                    for db in range(DB):
                        tps = psum(128, 128, BF)
                        nc.tensor.transpose(
                            tps, pb[:, db * 128:(db + 1) * 128], idb_s
                        )
                        ptT = work.tile([128, 128], BF, tag=f"ptT{db % 2}")
                        nc.vector.tensor_copy(ptT, tps)
                        nc.tensor.matmul(
                            dps, lhsT=ptT, rhs=pall_s[:, db * 16:(db + 1) * 16],
                            start=(db == 0), stop=(db == DB - 1),
                        )
                    nc.vector.tensor_copy(dots[eidx][:, tt * 16:(tt + 1) * 16], dps)

            do_commit(0)  # x_init is history entry 0 (and initial partial)

            def hist_entries(l, post):
                es = [(0, l > 0 or post)]  # (entry idx, doubled?)
                for j, cl in enumerate((2, 4, 6)):
                    if l > cl or (l == cl and post):
                        es.append((j + 1, False))
                return es

            def aggregate(l, pa, post):
                """Depth aggregation over hist+partial with projection column
                pa; returns 6 bf16 d-tiles [128, 1024] holding the transposed,
                rms-normed (and ln_s-scaled) result. Small ops are batched
                across all 8 t-tiles (column-grouped [128, TT*(m+1)])."""
                es = hist_entries(l, post)
                m = len(es)
                w = m + 1
                sl = float(ln_s[l])
                inv_sc = 1.0 / (768.0 * sl * sl)
                eps_sc = EPS / (sl * sl)
                qb = work.tile([128, D], F32, tag="qb")
                nc.gpsimd.dma_start(
                    out=qb, in_=pallt_d[pa:pa + 1, :].to_broadcast([128, D])
                )
                hT = [htp.tile([128, T], BF, tag=f"hT{db}", name=uname("hT"))
                      for db in range(DB)]
                # --- logits for all t-tiles at once: [128, tt, j] ---
                lg = small.tile([128, TT, w], F32, tag="lgB")
                sq = small.tile([128, TT, w], F32, tag="sqB")
                for j, (eidx, _dbl) in enumerate(es):
                    # cached dots at stride 16 -> [128, TT]
                    nc.vector.tensor_copy(
                        lg[:, :, j],
                        dots[eidx].rearrange("p (t c) -> p t c", c=16)[:, :, pa],
                    )
                    nc.vector.tensor_copy(sq[:, :, j], ssqs[eidx][:, :])
                for tt in range(TT):
                    dsl = dtp.tile([128, D], BF, tag="dtmpB", name=uname("dt"))
                    nc.vector.tensor_mul(dsl, pt[tt], qb)
                    nc.vector.tensor_reduce(
                        lg[:, tt, m:m + 1], dsl, axis=AX.X, op=ALU.add
                    )
                    trs = work.tile([128, D], BF, tag="trash")
                    nc.scalar.activation(
                        trs, pt[tt], ACTF.Square, bias=c_zero,
                        accum_out=sq[:, tt, m:m + 1],
                    )
                # r = 1/sqrt(ssq/768 + eps); logit = dot*r (+ln2 for doubles)
                rt = small.tile([128, TT, w], F32, tag="rtB")
                nc.scalar.activation(rt, sq, ACTF.Sqrt, scale=1.0 / 768.0,
                                     bias=c_eps)
                rr = small.tile([128, TT, w], F32, tag="rrB")
                nc.vector.reciprocal(rr, rt)
                lg2 = small.tile([128, TT, w], F32, tag="lg2B")
                nc.vector.tensor_mul(lg2, lg, rr)
                for j, (eidx, dbl) in enumerate(es):
                    if dbl:
                        nc.vector.tensor_scalar_add(lg2[:, :, j], lg2[:, :, j], LN2)
                ew = small.tile([128, TT, w], F32, tag="ewB")
                nc.scalar.activation(ew, lg2, ACTF.Exp, bias=c_zero)
                zz = small.tile([128, TT], F32, tag="zzB")
                nc.vector.tensor_reduce(zz, ew, axis=AX.X, op=ALU.add)
                zb = small.tile([128, TT], F32, tag="zbB")
                nc.vector.tensor_mul(zb, zz, zz)
                hs = small.tile([128, TT], F32, tag="hsB2")
                rh = small.tile([128, TT], F32, tag="rhB")
                ewb = small.tile([128, TT, w], BF, tag="ewbB")
                nc.vector.tensor_copy(ewb, ew)
                for tt in range(TT):
                    hes = []
                    for (eidx, _dbl) in es:
                        he = hep.tile([128, D], BF, tag="he")
                        nc.sync.dma_start(
                            out=he, in_=edram[eidx][tt * 128:(tt + 1) * 128, :]
                        )
                        hes.append(he)
                    pb = pbp.tile([128, D], BF, tag="pbf")
                    nc.vector.tensor_copy(pb, pt[tt])
                    h0 = psum(128, 512)
                    h1 = psum(128, 256)
                    vs = hes + [pb]
                    for j, vt in enumerate(vs):
                        dg = diagp.tile([128, 128], BF, tag="dg")
                        nc.vector.tensor_scalar_mul(dg, idb_s, ew[:, tt, j:j + 1])
                        nc.tensor.matmul(h0, lhsT=dg, rhs=vt[:, 0:512],
                                         start=(j == 0), stop=(j == len(vs) - 1))
                        nc.tensor.matmul(h1, lhsT=dg, rhs=vt[:, 512:768],
                                         start=(j == 0), stop=(j == len(vs) - 1))
                    hsA = small.tile([128, 1], F32, tag="hsA")
                    tr3 = work.tile([128, D], BF, tag="trash")
                    nc.scalar.activation(tr3[:, 0:512], h0, ACTF.Square,
                                         bias=c_zero, accum_out=hsA)
                    hsB = small.tile([128, 1], F32, tag="hsB")
                    tr4 = work.tile([128, D], BF, tag="trash")
                    nc.scalar.activation(tr4[:, 0:256], h1, ACTF.Square,
                                         bias=c_zero, accum_out=hsB)
                    nc.vector.tensor_add(hs[:, tt:tt + 1], hsA, hsB)
                    # arg = hs*inv + eps_sc*Z^2 ; rh = 1/sqrt(arg)
                    rh0 = small.tile([128, 1], F32, tag="rh0")
                    nc.vector.tensor_scalar(
                        out=rh0, in0=hs[:, tt:tt + 1], scalar1=inv_sc,
                        scalar2=None, op0=ALU.mult,
                    )
                    rh1 = small.tile([128, 1], F32, tag="rh1")
                    nc.vector.tensor_scalar(
                        out=rh1, in0=zb[:, tt:tt + 1], scalar1=eps_sc,
                        scalar2=None, op0=ALU.mult,
                    )
                    rh2 = small.tile([128, 1], F32, tag="rh2")
                    nc.vector.tensor_add(rh2, rh0, rh1)
                    rh3 = small.tile([128, 1], F32, tag="rh3")
                    nc.scalar.activation(rh3, rh2, ACTF.Sqrt, bias=c_zero)
                    nc.vector.reciprocal(rh[:, tt:tt + 1], rh3)
                    hn = work.tile([128, D], BF, tag="hn")
                    nc.scalar.activation(hn[:, 0:512], h0, ACTF.Copy,
                                         scale=rh[:, tt:tt + 1])
                    nc.scalar.activation(hn[:, 512:768], h1, ACTF.Copy,
                                         scale=rh[:, tt:tt + 1])
                    tp0 = psum(128, 512, BF)
                    tp1 = psum(128, 256, BF)
                    for db in range(DB):
                        dst = (tp0[:, (db % 4) * 128:(db % 4 + 1) * 128] if db < 4
                               else tp1[:, (db - 4) * 128:(db - 3) * 128])
                        nc.tensor.transpose(dst, hn[:, db * 128:(db + 1) * 128],
                                            idb_s)
                    for db in range(DB):
                        srcp = (tp0[:, (db % 4) * 128:(db % 4 + 1) * 128] if db < 4
                                else tp1[:, (db - 4) * 128:(db - 3) * 128])
                        nc.vector.tensor_copy(
                            hT[db][:, tt * 128:(tt + 1) * 128], srcp
                        )
                return hT

            def mha(l, hT):
                wv = []
                wo = []
                for db in range(DB):
                    wvt = wpool.tile([128, D], BF, tag=f"wv{db}")
                    nc.sync.dma_start(out=wvt, in_=wv_d[l, db * 128:(db + 1) * 128, :])
                    wv.append(wvt)
                    wot = wpool.tile([128, D], BF, tag=f"wo{db}")
                    nc.sync.dma_start(out=wot, in_=wo_d[l, db * 128:(db + 1) * 128, :])
                    wo.append(wot)
                aT = [atp.tile([128, T], BF, tag=f"aT{db}", name=uname("aT")) for db in range(DB)]
                # head groups of 4 to bound SBUF: qk projections + V + attention
                for g3 in range(3):
                    qct = [2 * g3, 2 * g3 + 1, D // 128 + 2 * g3, D // 128 + 2 * g3 + 1]
                    wqg = []
                    for db in range(DB):
                        wq = wpool.tile([128, 512], BF, tag=f"wqg{db}",
                                        name=uname("wqg"))
                        nc.sync.dma_start(
                            out=wq,
                            in_=wqk_d[l, db * 128:(db + 1) * 128,
                                      g3 * 512:(g3 + 1) * 512],
                        )
                        wqg.append(wq)
                    qkg = []
                    for li, ct in enumerate(qct):
                        # local column window inside wqg: q -> 0:256, k -> 256:512
                        lc0 = (li % 2) * 128 + (li // 2) * 256
                        qt = qkp.tile([128, T], BF, tag=f"qkg{li}")
                        qkg.append(qt)
                        for th in range(2):
                            qp = psum(128, 512)
                            for db in range(DB):
                                nc.tensor.matmul(
                                    qp, lhsT=wqg[db][:, lc0:lc0 + 128],
                                    rhs=hT[db][:, th * 512:(th + 1) * 512],
                                    start=(db == 0), stop=(db == DB - 1),
                                )
                            nc.vector.tensor_copy(qt[:, th * 512:(th + 1) * 512], qp)
                    # V columns for this head group (4 heads x 64) + ones cols
                    vag = []
                    for tt in range(TT):
                        va = vap.tile([128, 4 * 65], BF, tag=f"vag{tt}")
                        var = va.rearrange("p (h c) -> p h c", c=65)
                        nc.vector.memset(var[:, :, 64:65], 1.0)
                        vp = psum(128, 256)
                        for db in range(DB):
                            nc.tensor.matmul(
                                vp, lhsT=hT[db][:, tt * 128:(tt + 1) * 128],
                                rhs=wv[db][:, g3 * 256:(g3 + 1) * 256],
                                start=(db == 0), stop=(db == DB - 1),
                            )
                        for lh in range(4):
                            nc.vector.tensor_copy(
                                var[:, lh, 0:64], vp[:, lh * 64:(lh + 1) * 64]
                            )
                        vag.append(va)
                    for lh in range(4):
                        h = 4 * g3 + lh
                        qtile = qkg[lh // 2]
                        ktile = qkg[2 + lh // 2]
                        r0 = (lh % 2) * 64
                        eP = []
                        for kt in range(TT):
                            ep = expp.tile([128, T], BF, tag=f"eP{kt}")
                            eP.append(ep)
                            for qc in range(2):
                                cs = max(kt * 128, qc * 512)
                                ce = (qc + 1) * 512
                                if cs >= ce:
                                    continue
                                sp = psum(128, ce - cs)
                                nc.tensor.matmul(
                                    sp,
                                    lhsT=ktile[r0:r0 + 64, kt * 128:(kt + 1) * 128],
                                    rhs=qtile[r0:r0 + 64, cs:ce],
                                    start=True, stop=True,
                                )
                                if cs == kt * 128:
                                    nc.vector.tensor_add(
                                        sp[:, 0:128], sp[:, 0:128], mask_s
                                    )
                                nc.scalar.activation(
                                    ep[:, cs - kt * 128:ce - kt * 128], sp,
                                    ACTF.Exp, bias=c_zero,
                                )
                        for qc in range(2):
                            ap_ = psum(65, 512)
                            kts = [kt for kt in range(TT) if kt * 128 < (qc + 1) * 512]
                            for ji, kt in enumerate(kts):
                                cs = max(kt * 128, qc * 512)
                                ce = (qc + 1) * 512
                                nc.tensor.matmul(
                                    ap_[:, cs - qc * 512:512],
                                    lhsT=vag[kt][:, lh * 65:(lh + 1) * 65],
                                    rhs=eP[kt][:, cs - kt * 128:ce - kt * 128],
                                    start=(ji == 0), stop=(ji == len(kts) - 1),
                                )
                            rr = small.tile([1, 512], F32, tag="rrow")
                            nc.vector.reciprocal(rr, ap_[64:65, :])
                            rrd = drbp.tile([1, 512], F32, tag="rrd",
                                            name=uname("rrd"))
                            nc.sync.dma_start(out=rrd, in_=rr)
                            rb = work.tile([64, 512], BF, tag="rb")
                            nc.gpsimd.dma_start(
                                out=rb, in_=rrd.to_broadcast([64, 512])
                            )
                            nc.vector.tensor_mul(
                                aT[h // 2][r0:r0 + 64, qc * 512:(qc + 1) * 512],
                                ap_[0:64, :], rb,
                            )
                # output projection; on commit layers partial was zeroed first
                overwrite = (l % 2 == 0)
                for tt in range(TT):
                    for c0, cn in ((0, 512), (512, 256)):
                        wp = psum(128, cn)
                        for db in range(DB):
                            nc.tensor.matmul(
                                wp, lhsT=aT[db][:, tt * 128:(tt + 1) * 128],
                                rhs=wo[db][:, c0:c0 + cn],
                                start=(db == 0), stop=(db == DB - 1),
                            )
                        if overwrite:
                            nc.vector.tensor_copy(pt[tt][:, c0:c0 + cn], wp)
                        else:
                            nc.vector.tensor_add(
                                pt[tt][:, c0:c0 + cn], pt[tt][:, c0:c0 + cn], wp
                            )

            def mlp(l, hT):
                wm1 = []
                for db in range(DB):
                    wt = wpool.tile([128, 4 * D], BF, tag=f"wm1{db}")
                    nc.sync.dma_start(out=wt, in_=wm1_d[l, db * 128:(db + 1) * 128, :])
                    wm1.append(wt)
                for g in range(4):  # t-quarters, PSUM-resident output
                    mo = []
                    for ti in range(2):
                        mo.append((psum(128, 512), psum(128, 256)))
                    for ck in range(CK):
                        mp = psum(128, 256)
                        for db in range(DB):
                            nc.tensor.matmul(
                                mp, lhsT=wm1[db][:, ck * 128:(ck + 1) * 128],
                                rhs=hT[db][:, g * 256:(g + 1) * 256],
                                start=(db == 0), stop=(db == DB - 1),
                            )
                        m1 = m1p.tile([128, 256], BF, tag="m1")
                        nc.scalar.activation(m1, mp, ACTF.Gelu_apprx_tanh, bias=c_zero)
                        w2 = wm2p.tile([128, D], BF, tag="wm2")
                        nc.sync.dma_start(
                            out=w2, in_=wm2_d[l, ck * 128:(ck + 1) * 128, :]
                        )
                        for ti in range(2):
                            a, b = mo[ti]
                            nc.tensor.matmul(
                                a, lhsT=m1[:, ti * 128:(ti + 1) * 128],
                                rhs=w2[:, 0:512],
                                start=(ck == 0), stop=(ck == CK - 1),
                            )
                            nc.tensor.matmul(
                                b, lhsT=m1[:, ti * 128:(ti + 1) * 128],
                                rhs=w2[:, 512:768],
                                start=(ck == 0), stop=(ck == CK - 1),
                            )
                    for ti in range(2):
                        tt = 2 * g + ti
                        a, b = mo[ti]
                        nc.vector.tensor_add(pt[tt][:, 0:512], pt[tt][:, 0:512], a)
                        nc.vector.tensor_add(pt[tt][:, 512:768], pt[tt][:, 512:768], b)

            commit_slot = {2: 1, 4: 2, 6: 3}
            for l in range(L):
                hT1 = aggregate(l, l, post=False)
                if l in commit_slot:
                    do_commit(commit_slot[l])
                if "mha" not in _SKIP:
                    mha(l, hT1)
                if "agg2" not in _SKIP:
                    hT2 = aggregate(l, 8 + l, post=True)
                if "mlp" not in _SKIP:
                    mlp(l, hT2)

            for tt in range(TT):
                nc.sync.dma_start(out=out_d[tt * 128:(tt + 1) * 128, :], in_=pt[tt])

    lower_extended_insts(nc)
    _hoist_waits(nc)
    return nc


def _prep_host(inputs):
    bf = ml_dtypes.bfloat16
    x = np.asarray(inputs["x_init"], np.float32)
    Wqkv = np.asarray(inputs["Wqkv"], np.float32)
    Wo = np.asarray(inputs["Wo"], np.float32)
    Wm1 = np.asarray(inputs["Wm1"], np.float32)
    Wm2 = np.asarray(inputs["Wm2"], np.float32)
    attn_scale = np.asarray(inputs["attn_scale"], np.float32)
    mlp_scale = np.asarray(inputs["mlp_scale"], np.float32)
    apw = np.asarray(inputs["attn_proj_w"], np.float32)
    mpw = np.asarray(inputs["mlp_proj_w"], np.float32)
    ln_s = np.asarray(inputs["ln_s"], np.float32)

    wqk = Wqkv[:, :, : 2 * D].copy()
    wqk[:, :, :D] *= 1.0 / np.sqrt(HD)          # fold attention scale into Wq
    # device layout: per head-group g3, columns [q(g3*256:+256) | k(same)]
    wq_g = wqk[:, :, :D].reshape(L, D, 3, 256)
    wk_g = wqk[:, :, D:].reshape(L, D, 3, 256)
    wqk = np.concatenate([wq_g, wk_g], axis=3).reshape(L, D, 2 * D)
    wv = Wqkv[:, :, 2 * D:]
    wo = Wo * attn_scale[:, None, :]             # fold attn_scale into Wo cols
    wm2 = Wm2 * mlp_scale[:, None, :]            # fold mlp_scale into Wm2 cols
    pall = np.concatenate([apw.T, mpw.T], axis=1)  # [768, 16]
    # device layout: [128, db*16+col] so it loads in one DMA
    pall_dev = np.ascontiguousarray(
        pall.reshape(DB, 128, 16).transpose(1, 0, 2).reshape(128, DB * 16)
    )

    ki = np.arange(128)
    maskt = np.where(ki[:, None] <= ki[None, :], 0.0, -1e30).astype(np.float32)

    common = {
        "wqk": np.ascontiguousarray(wqk).astype(bf),
        "wv": np.ascontiguousarray(wv).astype(bf),
        "wo": np.ascontiguousarray(wo).astype(bf),
        "wm1": np.ascontiguousarray(Wm1).astype(bf),
        "wm2": np.ascontiguousarray(wm2).astype(bf),
        "pall": pall_dev.astype(bf),
        "pallt": np.ascontiguousarray(pall.T),
        "maskt": maskt,
        "idf": np.eye(128, dtype=np.float32),
        "idb": np.eye(128).astype(bf),
    }
    return x, common, tuple(float(v) for v in ln_s)


def kernel(**inputs):
    global LAST_RESULT
    x, common, ln_key = _prep_host(inputs)
    if ln_key not in _CACHE:
        _CACHE[ln_key] = build(ln_key)
    nc = _CACHE[ln_key]
    B = x.shape[0]
    in_maps = []
    for b in range(B):
        m = dict(common)
        m["x"] = np.ascontiguousarray(x[b])
        in_maps.append(m)
    res = run_bass_kernel_spmd(nc, in_maps, core_ids=list(range(B)))
    LAST_RESULT = res
    out = np.stack([res.results[b]["out"] for b in range(B)], axis=0)
    return out.astype(np.float32)

